# revision 1
# baseline (speedup 1.0000x reference)
"""LogNormCell kernel for 8 Trainium2 NeuronCores.

Math (per element): y = sigmoid(w[d] * ln(s) - q * ln(c) + bias)
  where s[b,t,d] = cumsum_t softplus(x[b,t,d]),  c = t+1.

Strategy:
  * Data-parallel over batch: 32 batches -> 4 per core, no cross-core comm.
  * Per core, tiles are [128 partitions = time within a block, 4*256 free =
    (batch, feature)]; 32 time blocks stream through a fused pipeline.
  * softplus = Ln(1 + Exp(x)); Exp and Ln both live in the single ACT table
    set `natural_log_exp_and_others`. Bacc's table-load pass alternates the
    per-function canonical sets, so a post-finalize fixup pins one load of
    the combined set and deletes the rest (saves ~90us of table DMAs).
  * cumsum along T via a triangular-ones matmul on the TensorEngine. The
    stationary is "rotated": out row 0 = block total, out row m = prefix
    through t_local=m-1. The running carry is folded in by adding the
    previous block's PSUM row 0 into row 0 of the next block's softplus tile
    (stationary row k=0 is all-ones, so the carry broadcasts to every row),
    keeping the carry chain partition-aligned for the VectorE.
  * sigmoid(z) = 1 / (1 + exp(-z)): Exp on ScalarE with the per-partition
    bias (q*ln(c) - bias) folded in; +1 and the fast reciprocal on VectorE.
  * Input in bf16 (cast on host): halves input HBM traffic; output f32.
  * The input pool has one buffer per block (no slot reuse) so every DMA
    needs at most one sync wait (the hardware's per-instruction limit);
    all DMAs are HWDGE (SWDGE descriptor generation costs ~2.5us/DMA).
"""

import numpy as np
import ml_dtypes

import concourse.bass as bass
import concourse.bacc as bacc_mod
import concourse.tile as tile
from concourse import mybir
from concourse.bass_utils import run_bass_kernel_spmd
from concourse.hw_specs import get_activation_tables

AF = mybir.ActivationFunctionType

B, T, D = 32, 4096, 256
NCORES = 8
BPC = B // NCORES          # batches per core
P = 128                    # partitions / time-block size
NBLK = T // P              # 32 time blocks
FREE = BPC * D             # 1024 free elements per tile

# Stash of the last run's BassKernelResults (exec_time_ns etc.) for harnesses
# that want timing; not needed for correctness.
LAST_RESULT = None

_CACHED_NC = None


def _dedup_act_table_loads(nc):
    """All activations here use only Exp/Ln, which share one table set.
    Keep a single load of the combined set; drop the alternating reloads."""
    tables = list(get_activation_tables(nc.m.arch).keys())
    combined = tables.index("natural_log_exp_and_others")
    for f in nc.m.functions:
        for block in f.blocks:
            loads = [
                i for i in block.instructions
                if type(i).__name__ == "InstLoadActFuncSet"
            ]
            if not loads:
                continue
            loads[0].act_func_set_id = combined
            for extra in loads[1:]:
                block.instructions.remove(extra)


def _build():
    nc = bacc_mod.Bacc()
    x = nc.declare_dram_parameter("x", [BPC, T, D], mybir.dt.bfloat16, isOutput=False)
    wb = nc.declare_dram_parameter("wb", [1, FREE], mybir.dt.float32, isOutput=False)
    bn = nc.declare_dram_parameter("bn", [P, NBLK], mybir.dt.float32, isOutput=False)
    ut = nc.declare_dram_parameter("ut", [P, P], mybir.dt.bfloat16, isOutput=False)
    y = nc.declare_dram_parameter("y", [BPC, T, D], mybir.dt.float32, isOutput=True)

    xv = x.rearrange("b (n p) d -> n p b d", p=P)   # [NBLK, 128, BPC, D]
    yv = y.rearrange("b (n p) d -> n p b d", p=P)

    with tile.TileContext(nc) as tc:
        with (
            tc.tile_pool(name="singles", bufs=1) as singles,
            tc.tile_pool(name="xin", bufs=NBLK) as xin_pool,
            tc.tile_pool(name="acc", bufs=4, space="PSUM") as psum_pool,
            tc.tile_pool(name="lg", bufs=4) as g_pool,
            tc.tile_pool(name="rr", bufs=4) as r_pool,
        ):
            Ut = singles.tile([P, P], mybir.dt.bfloat16)
            nc.sync.dma_start(out=Ut, in_=ut[:, :])
            WB = singles.tile([P, FREE], mybir.dt.float32)
            nc.sync.dma_start(out=WB, in_=wb[0:1, :].partition_broadcast(P))
            BN = singles.tile([P, NBLK], mybir.dt.float32)
            nc.sync.dma_start(out=BN, in_=bn[:, :])

            prev_psum = None
            for i in range(NBLK):
                xt = xin_pool.tile([P, BPC, D], mybir.dt.bfloat16)
                nc.sync.dma_start(out=xt, in_=xv[i])
                xf = xt.rearrange("p b d -> p (b d)")
                # u = exp(x); then softplus = ln(1 + u). Both in place (bf16).
                nc.scalar.activation(out=xf, in_=xf, func=AF.Exp)
                nc.scalar.activation(out=xf, in_=xf, func=AF.Ln, bias=1.0)
                if prev_psum is not None:
                    # Fold the running-sum carry into row 0: stationary row
                    # k=0 is all-ones, so this value joins every cumsum row of
                    # this block. The previous block's total lives on PSUM row
                    # 0 (rotated stationary) -> partition-aligned DVE add.
                    nc.vector.tensor_add(
                        out=xf[0:1, :],
                        in0=xf[0:1, :],
                        in1=prev_psum[0:1, :],
                    )
                # Rotated cumsum: psum row 0 = block total (t_local=127),
                # psum row m (m>=1) = prefix through t_local = m-1.
                ps = psum_pool.tile([P, FREE], mybir.dt.float32)
                nc.tensor.matmul(
                    ps[:, 0:512], Ut, xf[:, 0:512], start=True, stop=True
                )
                nc.tensor.matmul(
                    ps[:, 512:1024], Ut, xf[:, 512:1024], start=True, stop=True
                )
                # g = ln(s); h = w*g; e = exp(-h + (q ln c - bias)); e += 1.
                gt = g_pool.tile([P, FREE], mybir.dt.float32)
                nc.scalar.activation(out=gt, in_=ps, func=AF.Ln)
                nc.vector.tensor_mul(out=gt, in0=gt, in1=WB)
                nc.scalar.activation(
                    out=gt, in_=gt, func=AF.Exp, scale=-1.0, bias=BN[:, i : i + 1]
                )
                nc.vector.tensor_scalar_add(out=gt, in0=gt, scalar1=1.0)
                # y = 1/(1+e)
                rt = r_pool.tile([P, BPC, D], mybir.dt.float32)
                rf = rt.rearrange("p b d -> p (b d)")
                nc.vector.reciprocal_approx_fast(out=rf, in_=gt)
                # Undo the row rotation on the way out: tile rows 1..127 are
                # t_local 0..126; tile row 0 is t_local 127. SWDGE (gpsimd)
                # spreads store descriptors across all 16 SDMA engines; the
                # HWDGE dynamic queue funnels them through one.
                nc.gpsimd.dma_start(out=yv[i][0:127], in_=rt[1:128])
                nc.gpsimd.dma_start(out=yv[i][127:128], in_=rt[0:1])
                prev_psum = ps
    nc.finalize()
    _dedup_act_table_loads(nc)
    return nc


def kernel(inputs, w, q, bias):
    global LAST_RESULT, _CACHED_NC
    inputs = np.asarray(inputs, dtype=np.float32)
    w = np.asarray(w, dtype=np.float32)
    q = np.asarray(q, dtype=np.float32)
    bias = np.asarray(bias, dtype=np.float32)

    # Free axis of each tile is (b, d): tile w over the 4 local batches.
    wb = np.ascontiguousarray(np.tile(w[:, 0], BPC)[None, :])  # [1, FREE]
    # Rotated row order: psum row 0 holds t_local=127, row m holds t_local=m-1.
    # bn[m, i] = q * ln(c) - bias with c = t+1.
    c_mat = np.empty((P, NBLK), dtype=np.float64)
    blk = np.arange(NBLK, dtype=np.float64) * P
    c_mat[0, :] = blk + P
    c_mat[1:, :] = blk[None, :] + np.arange(1, P, dtype=np.float64)[:, None]
    bn = np.ascontiguousarray(
        (q[0, 0] * np.log(c_mat) - bias[0, 0]).astype(np.float32)
    )
    # Rotated cumsum stationary: out[m] = sum_{k<=m-1} rhs[k] for m>=1 and
    # out[0] = sum_k rhs[k] (the block total, used as the next carry).
    ut_np = np.triu(np.ones((P, P), np.float32), 1)
    ut_np[:, 0] = 1.0
    ut = ut_np.astype(ml_dtypes.bfloat16)

    if _CACHED_NC is None:
        _CACHED_NC = _build()
    nc = _CACHED_NC

    shards = inputs.astype(ml_dtypes.bfloat16).reshape(NCORES, BPC, T, D)
    in_maps = [
        {"x": np.ascontiguousarray(shards[i]), "wb": wb, "bn": bn, "ut": ut}
        for i in range(NCORES)
    ]
    res = run_bass_kernel_spmd(nc, in_maps, core_ids=list(range(NCORES)))
    LAST_RESULT = res
    out = np.stack([res.results[i]["y"] for i in range(NCORES)])
    return out.reshape(B, T, D)

